# revision 19
# baseline (speedup 1.0000x reference)
"""Causal multi-head attention (B=4, T=2048, D=1024, H=16, d_h=64) on 8 trn2 cores.

Sharding: data-parallel over batch (4) x tensor-parallel over head halves (2).
Core c handles batch c//2, heads [8*(c%2), 8*(c%2)+8), i.e. output columns
[512*(c%2), 512*(c%2)+512) of out[c//2].

v4 design — all matmul operands bf16; structured so the PE stays saturated
above the ACT exp floor (HAM clock-gates the PE to 1.2 GHz unless its duty
stays high, and re-warming needs a near-fully-busy 3.4us window):

  A) x host-cast to bf16, DMA'd TRANSPOSED via the DMA xbar into
     xT [128, 8*2048]; V projection (v natural, 128-wide per-head slices:
     [v | ones | zero-pad] so LDWEIGHTS gets FWL) overlaps chunk arrivals.
  B) qT/kT [128, 2048] per head-pair g (W stationary, accumulate 8 d_in
     tiles). Only g=0 runs as a phase; g=1..3 are INJECTED one matmul at a
     time into phase C's k-loops as PE filler above the ACT-bound stretches.
  C) per (g, j-block of 512 q, k-tile i), diag offset m = i-4j:
       scores st[:, 512sg+128m:] per head (truncated to causally live cols,
         no mask matmuls)
       pt = exp(st/8) on ACT, one strided ACTIVATE covering both heads
       triangular blocks of both heads *= T (0/1) in one strided GpSimd op
       ctxT[128, 512sg+128m:] += v128.T @ pt (PSUM accumulate, emitted two
       k-tiles late; rows 0-63 ctx, row 64 softmax denominator, rows 65-127
       zero pad)
     tails (cts cast -> PE transposes -> reciprocal -> normalize -> DMA) are
     chunked and paced into the following k-loop together with the injected
     B2 matmuls. Single pool scope end to end: no phase barriers, no PE
     drain, HAM stays warm.

PSUM: st ring 2x2 banks + ctxT 2 + nat 1 + proj ring 1 = 8 banks.
"""

import os
import sys

for _p in ("/opt/trn_rl_repo", "/root/.axon_site/_ro/trn_rl_repo"):
    if os.path.isdir(_p) and _p not in sys.path:
        sys.path.insert(0, _p)

import ml_dtypes
import numpy as np

import concourse.mybir as mybir  # noqa: E402
import concourse.tile as tile  # noqa: E402
from concourse import bacc  # noqa: E402
from concourse.bass_utils import run_bass_kernel_spmd  # noqa: E402

F32 = mybir.dt.float32
BF16 = mybir.dt.bfloat16
BF = ml_dtypes.bfloat16

P = 128
T = 2048
DIN = 1024
DL = 512          # local d_out per core
HL = 8            # local heads
DH = 64
NT = T // P       # 16 t-tiles
NDI = DIN // P    # 8 d_in tiles
SCALE = 1.0 / np.sqrt(DH)

Exp = mybir.ActivationFunctionType.Exp


def _build():
    nc = bacc.Bacc(None, target_bir_lowering=False)
    x = nc.dram_tensor("x", [T, DIN], BF16, kind="ExternalInput")
    wq = nc.dram_tensor("wq", [DIN, DL], BF16, kind="ExternalInput")
    wk = nc.dram_tensor("wk", [DIN, DL], BF16, kind="ExternalInput")
    wv = nc.dram_tensor("wv", [DIN, DL], BF16, kind="ExternalInput")
    ident_d = nc.dram_tensor("ident", [P, P], BF16, kind="ExternalInput")
    tmask_d = nc.dram_tensor("tmask", [P, 2 * P], BF16, kind="ExternalInput")
    out = nc.dram_tensor("out", [T, DL], F32, kind="ExternalOutput")

    w_r = {n: w[:].rearrange("(k p) n -> k p n", p=P) for n, w in
           (("q", wq), ("k", wk), ("v", wv))}
    # out rows 512*j + 128*c + p
    out_r = out[:].rearrange("(j c p) n -> j p c n", j=4, c=4)

    with tile.TileContext(nc) as tc:
        with (
            tc.tile_pool(name="const", bufs=1) as const,
            tc.tile_pool(name="qk", bufs=1) as qk_pool,
            tc.tile_pool(name="v", bufs=1) as v_pool,
            tc.tile_pool(name="xt", bufs=1) as xt_pool,
            tc.tile_pool(name="wvp", bufs=1) as wv_pool,
            tc.tile_pool(name="wqp", bufs=1) as wq_pool,
            tc.tile_pool(name="wkp", bufs=1) as wk_pool,
            tc.tile_pool(name="pt", bufs=6) as pt_pool,
            tc.tile_pool(name="cs", bufs=3) as cs_pool,
            tc.tile_pool(name="o", bufs=3) as o_pool,
            tc.tile_pool(name="ps_pj", bufs=1, space="PSUM") as ps_pj,
            tc.tile_pool(name="ps_s", bufs=2, space="PSUM") as ps_s,
            tc.tile_pool(name="ps_ctx", bufs=1, space="PSUM") as ps_ctx,
            tc.tile_pool(name="ps_nat", bufs=1, space="PSUM") as ps_nat,
        ):
            ident = const.tile([P, P], BF16)
            nc.sync.dma_start(out=ident, in_=ident_d[:])
            tmask2 = const.tile([P, 2 * P], BF16)
            nc.sync.dma_start(out=tmask2, in_=tmask_d[:])
            ones_bf = const.tile([P, HL], BF16)
            nc.vector.memset(ones_bf, 1.0)
            # v tiles: per head a 128-wide slice [v(64) | ones(1) | pad0(63)]
            # (128-wide stationary => FWL fast weight load)
            v_sb = [v_pool.tile([P, HL * P], BF16, tag=f"v{t_}",
                                name=f"v{t_}") for t_ in range(NT)]
            qTs = [qk_pool.tile([P, T], BF16, tag=f"qT{g}", name=f"qT{g}")
                   for g in range(4)]
            kTs = [qk_pool.tile([P, T], BF16, tag=f"kT{g}", name=f"kT{g}")
                   for g in range(4)]
            xT = xt_pool.tile([P, NDI * T], BF16, name="xT")
            wv_t = [wv_pool.tile([P, DL], BF16, tag=f"wv{di}",
                                 name=f"wv{di}") for di in range(NDI)]
            wq_t = [wq_pool.tile([P, DL], BF16, tag=f"wq{di}",
                                 name=f"wq{di}") for di in range(NDI)]
            wk_t = [wk_pool.tile([P, DL], BF16, tag=f"wk{di}",
                                 name=f"wk{di}") for di in range(NDI)]

            # ---- DMAs ----
            # all of x through the xbar transpose unit on the sync queue
            # (single-queue: the xbar is not safe from two queues);
            # weights go on the scalar queue so they land within ~10us.
            for di in range(NDI):
                nc.scalar.dma_start(out=wv_t[di], in_=w_r["v"][di])
            for tq in range(4):
                for di in range(NDI):
                    nc.sync.dma_start(
                        out=xT[:, T * di + 512 * tq:T * di + 512 * (tq + 1)],
                        in_=x[:][512 * tq:512 * (tq + 1),
                                 P * di:P * (di + 1)],
                        transpose=True)
            for di in range(NDI):
                nc.scalar.dma_start(out=wq_t[di], in_=w_r["q"][di])
            for di in range(NDI):
                nc.scalar.dma_start(out=wk_t[di], in_=w_r["k"][di])

            # ---- Phase A: V projection (DMA-paced) ----
            for t_ in range(NT):
                psv = ps_pj.tile([P, DL], F32, tag="pj", name="psv")
                for di in range(NDI):
                    nc.tensor.matmul(
                        psv, xT[:, T * di + P * t_:T * di + P * (t_ + 1)],
                        wv_t[di], start=(di == 0), stop=(di == NDI - 1))
                vt = v_sb[t_][:].rearrange("p (h e) -> p h e", e=P)
                nc.vector.tensor_copy(
                    vt[:, :, 0:DH], psv[:].rearrange("p (h d) -> p h d", d=DH))
                nc.vector.tensor_copy(vt[:, :, DH], ones_bf)
                nc.vector.memset(vt[:, :, DH + 1:P], 0.0)

            # ---- B2 projection emitters (g=0 inline; g>=1 injected) ----
            def b2_chunks(g, tbs=range(4)):
                chunks = []
                for tb in tbs:
                    for which, w_t, dst in (("q", wq_t, qTs[g]),
                                            ("k", wk_t, kTs[g])):
                        state = {}

                        def mm(di, state=state, w_t=w_t, g=g, tb=tb):
                            if di == 0:
                                state["ps"] = ps_pj.tile([P, DL], F32,
                                                         tag="pj", name="pj")
                            nc.tensor.matmul(
                                state["ps"], w_t[di][:, P * g:P * (g + 1)],
                                xT[:, T * di + 512 * tb:
                                   T * di + 512 * (tb + 1)],
                                start=(di == 0), stop=(di == NDI - 1))

                        def cp(state=state, dst=dst, tb=tb):
                            nc.vector.tensor_copy(
                                dst[:, 512 * tb:512 * (tb + 1)], state["ps"])

                        for di in range(NDI):
                            chunks.append(lambda di=di, mm=mm: mm(di))
                        chunks.append(cp)
                        # spacer: give the copy a tile of headroom before the
                        # next group's first matmul needs the PSUM slot back
                        chunks.append(lambda: None)
                return chunks

            for c in b2_chunks(0):
                c()

            # ---- Phase C ----
            Copy = mybir.ActivationFunctionType.Copy

            def tail_chunks(g, j, ctxT, use_act=False):
                chunks = []
                half = {}

                def c_cts(sg):
                    cts = cs_pool.tile([DH + 2, 512], BF16, tag="cts",
                                       name="cts")
                    src_ = ctxT[0:DH + 2, 512 * sg:512 * (sg + 1)]
                    if use_act and sg == 1:
                        nc.scalar.copy(cts, src_)
                    else:
                        nc.vector.tensor_copy(cts, src_)
                    half[sg] = {"cts": cts}

                def c_tr(sg, lo):
                    st_ = half[sg]
                    if "natall" not in half:
                        half["natall"] = ps_nat.tile([P, 8 * (DH + 2)], BF16,
                                                     tag="nat", name="nat")
                    if "nat" not in st_:
                        na = half["natall"]
                        st_["nat"] = na[:, 4 * (DH + 2) * sg:
                                        4 * (DH + 2) * (sg + 1)]
                    nat, cts = st_["nat"], st_["cts"]
                    for c in (lo, lo + 1):
                        nc.tensor.transpose(
                            nat[:, (DH + 2) * c:(DH + 2) * (c + 1)],
                            cts[0:DH + 2, P * c:P * (c + 1)],
                            ident[0:DH + 2, 0:DH + 2])

                def c_rec(sg):
                    st_ = half[sg]
                    rec = o_pool.tile([P, 4], F32, tag="rec", name="rec")
                    nc.vector.reciprocal(
                        rec, st_["nat"][:].rearrange(
                            "p (c e) -> p c e", e=DH + 2)[:, :, DH])
                    st_["rec"] = rec
                    st_["ob"] = o_pool.tile([P, 4 * DH], F32, tag="ob",
                                            name="ob")

                def c_norm(sg, lo):
                    st_ = half[sg]
                    for c in (lo, lo + 1):
                        dst = st_["ob"][:, DH * c:DH * (c + 1)]
                        src_ = st_["nat"][:, (DH + 2) * c:(DH + 2) * c + DH]
                        if use_act and sg == 1:
                            nc.scalar.activation(dst, src_, Copy,
                                                 scale=st_["rec"][:, c:c + 1])
                        else:
                            nc.vector.tensor_scalar_mul(
                                dst, src_, st_["rec"][:, c:c + 1])

                def c_out(sg):
                    h = 2 * g + sg
                    nc.sync.dma_start(
                        out=out_r[j][:, :, DH * h:DH * (h + 1)],
                        in_=half[sg]["ob"][:].rearrange("p (c d) -> p c d",
                                                        d=DH))

                chunks += [lambda: c_cts(0), lambda: c_cts(1)]
                for sg in range(2):
                    chunks += [
                        lambda sg=sg: c_tr(sg, 0),
                        lambda sg=sg: c_tr(sg, 2),
                        lambda sg=sg: c_rec(sg),
                        lambda sg=sg: c_norm(sg, 0),
                        lambda sg=sg: c_norm(sg, 2),
                        lambda sg=sg: c_out(sg),
                    ]
                return chunks

            pending = []     # tail chunks of the previous (g, j)
            av_q = []
            carry = None
            for g in range(4):
                b2_pend = b2_chunks(g + 1) if g < 3 else []
                n_tiles_left = sum(4 * j + 4 for j in range(4))
                for j in (0, 1, 2, 3):
                    nk = 4 * j + 4
                    ctxT = ps_ctx.tile([P, 1024], F32, tag="cT", name="ctxT")
                    for i in range(nk):
                        m = i - 4 * j
                        c0 = P * m if m > 0 else 0
                        st = ps_s.tile([P, 1024], F32, tag="s", name="st")
                        for sg in range(2):
                            nc.tensor.matmul(
                                st[:, 512 * sg + c0:512 * (sg + 1)],
                                kTs[g][DH * sg:DH * (sg + 1),
                                       P * i:P * (i + 1)],
                                qTs[g][DH * sg:DH * (sg + 1),
                                       512 * j + c0:512 * (j + 1)],
                                start=True, stop=True)
                        pt = pt_pool.tile([P, 1024], BF16, tag="pt",
                                          name="pt")
                        if m < 0:
                            nc.scalar.activation(pt[:, 0:1024], st[:, 0:1024],
                                                 Exp, scale=float(SCALE))
                        else:
                            nc.scalar.activation(
                                pt[:].rearrange("p (s q) -> p s q",
                                                s=2)[:, :, c0:512],
                                st[:].rearrange("p (s q) -> p s q",
                                                s=2)[:, :, c0:512],
                                Exp, scale=float(SCALE))
                        if m >= 0:
                            pm = pt[:].rearrange("p (s q) -> p s q",
                                                 s=2)[:, :, c0:c0 + P]
                            nc.vector.tensor_mul(
                                pm, pm,
                                tmask2[:].rearrange("p (s q) -> p s q", s=2))
                        if carry is not None:
                            carry()
                            carry = None
                        if pending:
                            nflush = -(-len(pending) // (nk - i))
                            for _ in range(nflush):
                                pending.pop(0)()
                        if b2_pend:
                            nb = -(-len(b2_pend) // max(n_tiles_left, 1))
                            for _ in range(min(nb, len(b2_pend))):
                                b2_pend.pop(0)()

                        def av(i=i, pt=pt, ctxT=ctxT, nk=nk, g=g, c0=c0):
                            for sg in range(2):
                                h = 2 * g + sg
                                nc.tensor.matmul(
                                    ctxT[:, 512 * sg + c0:512 * (sg + 1)],
                                    v_sb[i][:, P * h:P * (h + 1)],
                                    pt[:, 512 * sg + c0:512 * (sg + 1)],
                                    start=(i == 0), stop=(i == nk - 1),
                                    skip_group_check=True)
                        av_q.append(av)
                        if len(av_q) > 3:
                            av_q.pop(0)()
                        n_tiles_left -= 1
                    while len(av_q) > 1:
                        av_q.pop(0)()
                    carry = av_q.pop(0)
                    for c in pending:
                        c()
                    pending = tail_chunks(g, j, ctxT,
                                          use_act=(g == 3 and j == 3))
                for c in b2_pend:
                    c()
            if carry is not None:
                carry()
            for c in pending:
                c()
    nc.compile()
    return nc


_NC = None


def _get_nc():
    global _NC
    if _NC is None:
        _NC = _build()
    return _NC


_IDENT = np.eye(P, dtype=np.float32).astype(BF)
# T[p, u] = 1 where in-block column u (query) >= partition p (key); two
# copies side by side so one strided gpsimd op masks both heads
_T1 = (np.arange(P)[None, :] >= np.arange(P)[:, None]).astype(np.float32)
_TMASK = np.concatenate([_T1, _T1], axis=1).astype(BF)


def run(inputs, **spmd_kwargs):
    x, W_q, W_k, W_v = (inputs["x"], inputs["W_q"], inputs["W_k"], inputs["W_v"])
    nc = _get_nc()
    in_maps = []
    for c in range(8):
        b, half = divmod(c, 2)
        sl = slice(DL * half, DL * half + DL)
        in_maps.append({
            "x": np.ascontiguousarray(np.asarray(x[b]).astype(BF)),
            "wq": np.ascontiguousarray(np.asarray(W_q[:, sl]).astype(BF)),
            "wk": np.ascontiguousarray(np.asarray(W_k[:, sl]).astype(BF)),
            "wv": np.ascontiguousarray(np.asarray(W_v[:, sl]).astype(BF)),
            "ident": _IDENT,
            "tmask": _TMASK,
        })
    res = run_bass_kernel_spmd(nc, in_maps, core_ids=list(range(8)), **spmd_kwargs)
    B = x.shape[0]
    full = np.empty((B, T, 2 * DL), dtype=np.float32)
    for c in range(8):
        b, half = divmod(c, 2)
        full[b][:, DL * half:DL * half + DL] = res.results[c]["out"]
    return full, res


def kernel(**inputs):
    return run(inputs)[0]


if __name__ == "__main__":
    rng = np.random.default_rng(0)
    ins = {
        "x": rng.standard_normal((4, T, DIN), dtype=np.float32),
        "W_q": (rng.random((DIN, 2 * DL), dtype=np.float32) - 0.5) / 16,
        "W_k": (rng.random((DIN, 2 * DL), dtype=np.float32) - 0.5) / 16,
        "W_v": (rng.random((DIN, 2 * DL), dtype=np.float32) - 0.5) / 16,
    }
    o = kernel(**ins)
    print("ran ok", o.shape, o.dtype)


# revision 20
# speedup vs baseline: 1.2297x; 1.2297x over previous
"""Causal multi-head attention (B=4, T=2048, D=1024, H=16, d_h=64) on 8 trn2 cores.

Sharding: data-parallel over batch (4) x tensor-parallel over head halves (2).
Core c handles batch c//2, heads [8*(c%2), 8*(c%2)+8), i.e. output columns
[512*(c%2), 512*(c%2)+512) of out[c//2].

v4 design — all matmul operands bf16; structured so the PE stays saturated
above the ACT exp floor (HAM clock-gates the PE to 1.2 GHz unless its duty
stays high, and re-warming needs a near-fully-busy 3.4us window):

  A) x host-cast to bf16, DMA'd TRANSPOSED via the DMA xbar into
     xT [128, 8*2048]; V projection (v natural, 128-wide per-head slices:
     [v | ones | zero-pad] so LDWEIGHTS gets FWL) overlaps chunk arrivals.
  B) qT/kT [128, 2048] per head-pair g (W stationary, accumulate 8 d_in
     tiles). Only g=0 runs as a phase; g=1..3 are INJECTED one matmul at a
     time into phase C's k-loops as PE filler above the ACT-bound stretches.
  C) per (g, j-block of 512 q, k-tile i), diag offset m = i-4j:
       scores st[:, 512sg+128m:] per head (truncated to causally live cols,
         no mask matmuls)
       pt = exp(st/8) on ACT, one strided ACTIVATE covering both heads
       triangular blocks of both heads *= T (0/1) in one strided GpSimd op
       ctxT[128, 512sg+128m:] += v128.T @ pt (PSUM accumulate, emitted two
       k-tiles late; rows 0-63 ctx, row 64 softmax denominator, rows 65-127
       zero pad)
     tails (cts cast -> PE transposes -> reciprocal -> normalize -> DMA) are
     chunked and paced into the following k-loop together with the injected
     B2 matmuls. Single pool scope end to end: no phase barriers, no PE
     drain, HAM stays warm.

PSUM: st ring 2x2 banks + ctxT 2 + nat 1 + proj ring 1 = 8 banks.
"""

import os
import sys

for _p in ("/opt/trn_rl_repo", "/root/.axon_site/_ro/trn_rl_repo"):
    if os.path.isdir(_p) and _p not in sys.path:
        sys.path.insert(0, _p)

import ml_dtypes
import numpy as np

import concourse.mybir as mybir  # noqa: E402
import concourse.tile as tile  # noqa: E402
from concourse import bacc  # noqa: E402
from concourse.bass_utils import run_bass_kernel_spmd  # noqa: E402

F32 = mybir.dt.float32
BF16 = mybir.dt.bfloat16
BF = ml_dtypes.bfloat16

P = 128
T = 2048
DIN = 1024
DL = 512          # local d_out per core
HL = 8            # local heads
DH = 64
NT = T // P       # 16 t-tiles
NDI = DIN // P    # 8 d_in tiles
SCALE = 1.0 / np.sqrt(DH)

Exp = mybir.ActivationFunctionType.Exp


def _build():
    nc = bacc.Bacc(None, target_bir_lowering=False)
    x = nc.dram_tensor("x", [T, DIN], BF16, kind="ExternalInput")
    wq = nc.dram_tensor("wq", [DIN, DL], BF16, kind="ExternalInput")
    wk = nc.dram_tensor("wk", [DIN, DL], BF16, kind="ExternalInput")
    wv = nc.dram_tensor("wv", [DIN, DL], BF16, kind="ExternalInput")
    ident_d = nc.dram_tensor("ident", [P, P], BF16, kind="ExternalInput")
    tmask_d = nc.dram_tensor("tmask", [P, 2 * P], BF16, kind="ExternalInput")
    out = nc.dram_tensor("out", [T, DL], F32, kind="ExternalOutput")

    w_r = {n: w[:].rearrange("(k p) n -> k p n", p=P) for n, w in
           (("q", wq), ("k", wk), ("v", wv))}
    # out rows 512*j + 128*c + p
    out_r = out[:].rearrange("(j c p) n -> j p c n", j=4, c=4)

    with tile.TileContext(nc) as tc:
        with (
            tc.tile_pool(name="const", bufs=1) as const,
            tc.tile_pool(name="qk", bufs=1) as qk_pool,
            tc.tile_pool(name="v", bufs=1) as v_pool,
            tc.tile_pool(name="xt", bufs=1) as xt_pool,
            tc.tile_pool(name="wvp", bufs=1) as wv_pool,
            tc.tile_pool(name="wqp", bufs=1) as wq_pool,
            tc.tile_pool(name="wkp", bufs=1) as wk_pool,
            tc.tile_pool(name="pt", bufs=6) as pt_pool,
            tc.tile_pool(name="cs", bufs=3) as cs_pool,
            tc.tile_pool(name="o", bufs=3) as o_pool,
            tc.tile_pool(name="ps_pj", bufs=1, space="PSUM") as ps_pj,
            tc.tile_pool(name="ps_s", bufs=2, space="PSUM") as ps_s,
            tc.tile_pool(name="ps_ctx", bufs=1, space="PSUM") as ps_ctx,
            tc.tile_pool(name="ps_nat", bufs=1, space="PSUM") as ps_nat,
        ):
            ident = const.tile([P, P], BF16)
            nc.sync.dma_start(out=ident, in_=ident_d[:])
            tmask2 = const.tile([P, 2 * P], BF16)
            nc.sync.dma_start(out=tmask2, in_=tmask_d[:])
            ones_bf = const.tile([P, HL], BF16)
            nc.vector.memset(ones_bf, 1.0)
            # v tiles: per head a 128-wide slice [v(64) | ones(1) | pad0(63)]
            # (128-wide stationary => FWL fast weight load)
            v_sb = [v_pool.tile([P, HL * P], BF16, tag=f"v{t_}",
                                name=f"v{t_}") for t_ in range(NT)]
            qTs = [qk_pool.tile([P, T], BF16, tag=f"qT{g}", name=f"qT{g}")
                   for g in range(4)]
            kTs = [qk_pool.tile([P, T], BF16, tag=f"kT{g}", name=f"kT{g}")
                   for g in range(4)]
            xT = xt_pool.tile([P, NDI * T], BF16, name="xT")
            wv_t = [wv_pool.tile([P, DL], BF16, tag=f"wv{di}",
                                 name=f"wv{di}") for di in range(NDI)]
            wq_t = [wq_pool.tile([P, DL], BF16, tag=f"wq{di}",
                                 name=f"wq{di}") for di in range(NDI)]
            wk_t = [wk_pool.tile([P, DL], BF16, tag=f"wk{di}",
                                 name=f"wk{di}") for di in range(NDI)]

            # ---- DMAs ----
            # all of x through the xbar transpose unit on the sync queue
            # (single-queue: the xbar is not safe from two queues);
            # weights go on the scalar queue so they land within ~10us.
            for di in range(NDI):
                nc.sync.dma_start(out=wv_t[di], in_=w_r["v"][di])
            for tq in range(4):
                for di in range(NDI):
                    nc.sync.dma_start(
                        out=xT[:, T * di + 512 * tq:T * di + 512 * (tq + 1)],
                        in_=x[:][512 * tq:512 * (tq + 1),
                                 P * di:P * (di + 1)],
                        transpose=True)
            for di in range(NDI):
                nc.sync.dma_start(out=wq_t[di], in_=w_r["q"][di])
            for di in range(NDI):
                nc.sync.dma_start(out=wk_t[di], in_=w_r["k"][di])

            # ---- Phase A: V projection (DMA-paced) ----
            for t_ in range(NT):
                psv = ps_pj.tile([P, DL], F32, tag="pj", name="psv")
                for di in range(NDI):
                    nc.tensor.matmul(
                        psv, xT[:, T * di + P * t_:T * di + P * (t_ + 1)],
                        wv_t[di], start=(di == 0), stop=(di == NDI - 1))
                vt = v_sb[t_][:].rearrange("p (h e) -> p h e", e=P)
                nc.vector.tensor_copy(
                    vt[:, :, 0:DH], psv[:].rearrange("p (h d) -> p h d", d=DH))
                nc.vector.tensor_copy(vt[:, :, DH], ones_bf)
                nc.vector.memset(vt[:, :, DH + 1:P], 0.0)

            # ---- B2 projection emitters (g=0 inline; g>=1 injected) ----
            def b2_chunks(g):
                chunks = []
                for which, w_t, dst in (("q", wq_t, qTs[g]),
                                        ("k", wk_t, kTs[g])):
                    for tb in range(4):
                        state = {}

                        def mm(di, state=state, w_t=w_t, g=g, tb=tb):
                            if di == 0:
                                state["ps"] = ps_pj.tile([P, DL], F32,
                                                         tag="pj", name="pj")
                            nc.tensor.matmul(
                                state["ps"], w_t[di][:, P * g:P * (g + 1)],
                                xT[:, T * di + 512 * tb:
                                   T * di + 512 * (tb + 1)],
                                start=(di == 0), stop=(di == NDI - 1))

                        def cp(state=state, dst=dst, tb=tb):
                            nc.vector.tensor_copy(
                                dst[:, 512 * tb:512 * (tb + 1)], state["ps"])

                        for di in range(NDI):
                            chunks.append(lambda di=di, mm=mm: mm(di))
                        chunks.append(cp)
                        # spacer: give the copy a tile of headroom before the
                        # next group's first matmul needs the PSUM slot back
                        chunks.append(lambda: None)
                return chunks

            for c in b2_chunks(0):
                c()

            # ---- Phase C ----
            Copy = mybir.ActivationFunctionType.Copy

            def tail_chunks(g, j, ctxT, use_act=False):
                chunks = []
                half = {}

                def c_cts(sg):
                    cts = cs_pool.tile([DH + 2, 512], BF16, tag="cts",
                                       name="cts")
                    src_ = ctxT[0:DH + 2, 512 * sg:512 * (sg + 1)]
                    if use_act and sg == 1:
                        nc.scalar.copy(cts, src_)
                    else:
                        nc.vector.tensor_copy(cts, src_)
                    half[sg] = {"cts": cts}

                def c_tr(sg, lo):
                    st_ = half[sg]
                    if "natall" not in half:
                        half["natall"] = ps_nat.tile([P, 8 * (DH + 2)], BF16,
                                                     tag="nat", name="nat")
                    if "nat" not in st_:
                        na = half["natall"]
                        st_["nat"] = na[:, 4 * (DH + 2) * sg:
                                        4 * (DH + 2) * (sg + 1)]
                    nat, cts = st_["nat"], st_["cts"]
                    for c in (lo, lo + 1):
                        nc.tensor.transpose(
                            nat[:, (DH + 2) * c:(DH + 2) * (c + 1)],
                            cts[0:DH + 2, P * c:P * (c + 1)],
                            ident[0:DH + 2, 0:DH + 2])

                def c_rec(sg):
                    st_ = half[sg]
                    rec = o_pool.tile([P, 4], F32, tag="rec", name="rec")
                    nc.vector.reciprocal(
                        rec, st_["nat"][:].rearrange(
                            "p (c e) -> p c e", e=DH + 2)[:, :, DH])
                    st_["rec"] = rec
                    st_["ob"] = o_pool.tile([P, 4 * DH], F32, tag="ob",
                                            name="ob")

                def c_norm(sg, lo):
                    st_ = half[sg]
                    for c in (lo, lo + 1):
                        dst = st_["ob"][:, DH * c:DH * (c + 1)]
                        src_ = st_["nat"][:, (DH + 2) * c:(DH + 2) * c + DH]
                        if use_act and sg == 1:
                            nc.scalar.activation(dst, src_, Copy,
                                                 scale=st_["rec"][:, c:c + 1])
                        else:
                            nc.vector.tensor_scalar_mul(
                                dst, src_, st_["rec"][:, c:c + 1])

                def c_out(sg):
                    h = 2 * g + sg
                    nc.sync.dma_start(
                        out=out_r[j][:, :, DH * h:DH * (h + 1)],
                        in_=half[sg]["ob"][:].rearrange("p (c d) -> p c d",
                                                        d=DH))

                chunks += [lambda: c_cts(0), lambda: c_cts(1)]
                for sg in range(2):
                    chunks += [
                        lambda sg=sg: c_tr(sg, 0),
                        lambda sg=sg: c_tr(sg, 2),
                        lambda sg=sg: c_rec(sg),
                        lambda sg=sg: c_norm(sg, 0),
                        lambda sg=sg: c_norm(sg, 2),
                        lambda sg=sg: c_out(sg),
                    ]
                return chunks

            pending = []     # tail chunks of the previous (g, j)
            av_q = []
            carry = None
            for g in range(4):
                b2_pend = b2_chunks(g + 1) if g < 3 else []
                n_tiles_left = sum(4 * j + 4 for j in (3, 0, 2, 1))
                for j in (3, 0, 2, 1):
                    nk = 4 * j + 4
                    ctxT = ps_ctx.tile([P, 1024], F32, tag="cT", name="ctxT")
                    for i in range(nk):
                        m = i - 4 * j
                        c0 = P * m if m > 0 else 0
                        st = ps_s.tile([P, 1024], F32, tag="s", name="st")
                        for sg in range(2):
                            nc.tensor.matmul(
                                st[:, 512 * sg + c0:512 * (sg + 1)],
                                kTs[g][DH * sg:DH * (sg + 1),
                                       P * i:P * (i + 1)],
                                qTs[g][DH * sg:DH * (sg + 1),
                                       512 * j + c0:512 * (j + 1)],
                                start=True, stop=True)
                        pt = pt_pool.tile([P, 1024], BF16, tag="pt",
                                          name="pt")
                        if m < 0:
                            nc.scalar.activation(pt[:, 0:1024], st[:, 0:1024],
                                                 Exp, scale=float(SCALE))
                        else:
                            nc.scalar.activation(
                                pt[:].rearrange("p (s q) -> p s q",
                                                s=2)[:, :, c0:512],
                                st[:].rearrange("p (s q) -> p s q",
                                                s=2)[:, :, c0:512],
                                Exp, scale=float(SCALE))
                        if m >= 0:
                            pm = pt[:].rearrange("p (s q) -> p s q",
                                                 s=2)[:, :, c0:c0 + P]
                            nc.vector.tensor_mul(
                                pm, pm,
                                tmask2[:].rearrange("p (s q) -> p s q", s=2))
                        if carry is not None:
                            carry()
                            carry = None
                        if pending:
                            nflush = -(-len(pending) // (nk - i))
                            for _ in range(nflush):
                                pending.pop(0)()
                        if b2_pend:
                            nb = -(-len(b2_pend) // max(n_tiles_left, 1))
                            for _ in range(min(nb, len(b2_pend))):
                                b2_pend.pop(0)()

                        def av(i=i, pt=pt, ctxT=ctxT, nk=nk, g=g, c0=c0):
                            for sg in range(2):
                                h = 2 * g + sg
                                nc.tensor.matmul(
                                    ctxT[:, 512 * sg + c0:512 * (sg + 1)],
                                    v_sb[i][:, P * h:P * (h + 1)],
                                    pt[:, 512 * sg + c0:512 * (sg + 1)],
                                    start=(i == 0), stop=(i == nk - 1),
                                    skip_group_check=True)
                        av_q.append(av)
                        if len(av_q) > 3:
                            av_q.pop(0)()
                        n_tiles_left -= 1
                    while len(av_q) > 1:
                        av_q.pop(0)()
                    carry = av_q.pop(0)
                    for c in pending:
                        c()
                    pending = tail_chunks(g, j, ctxT,
                                          use_act=(g == 3 and j == 1))
                for c in b2_pend:
                    c()
            if carry is not None:
                carry()
            for c in pending:
                c()
    nc.compile()
    return nc


_NC = None


def _get_nc():
    global _NC
    if _NC is None:
        _NC = _build()
    return _NC


_IDENT = np.eye(P, dtype=np.float32).astype(BF)
# T[p, u] = 1 where in-block column u (query) >= partition p (key); two
# copies side by side so one strided gpsimd op masks both heads
_T1 = (np.arange(P)[None, :] >= np.arange(P)[:, None]).astype(np.float32)
_TMASK = np.concatenate([_T1, _T1], axis=1).astype(BF)


def run(inputs, **spmd_kwargs):
    x, W_q, W_k, W_v = (inputs["x"], inputs["W_q"], inputs["W_k"], inputs["W_v"])
    nc = _get_nc()
    in_maps = []
    for c in range(8):
        b, half = divmod(c, 2)
        sl = slice(DL * half, DL * half + DL)
        in_maps.append({
            "x": np.ascontiguousarray(np.asarray(x[b]).astype(BF)),
            "wq": np.ascontiguousarray(np.asarray(W_q[:, sl]).astype(BF)),
            "wk": np.ascontiguousarray(np.asarray(W_k[:, sl]).astype(BF)),
            "wv": np.ascontiguousarray(np.asarray(W_v[:, sl]).astype(BF)),
            "ident": _IDENT,
            "tmask": _TMASK,
        })
    res = run_bass_kernel_spmd(nc, in_maps, core_ids=list(range(8)), **spmd_kwargs)
    B = x.shape[0]
    full = np.empty((B, T, 2 * DL), dtype=np.float32)
    for c in range(8):
        b, half = divmod(c, 2)
        full[b][:, DL * half:DL * half + DL] = res.results[c]["out"]
    return full, res


def kernel(**inputs):
    return run(inputs)[0]


if __name__ == "__main__":
    rng = np.random.default_rng(0)
    ins = {
        "x": rng.standard_normal((4, T, DIN), dtype=np.float32),
        "W_q": (rng.random((DIN, 2 * DL), dtype=np.float32) - 0.5) / 16,
        "W_k": (rng.random((DIN, 2 * DL), dtype=np.float32) - 0.5) / 16,
        "W_v": (rng.random((DIN, 2 * DL), dtype=np.float32) - 0.5) / 16,
    }
    o = kernel(**ins)
    print("ran ok", o.shape, o.dtype)


# revision 21
# speedup vs baseline: 1.2362x; 1.0053x over previous
"""Causal multi-head attention (B=4, T=2048, D=1024, H=16, d_h=64) on 8 trn2 cores.

Sharding: data-parallel over batch (4) x tensor-parallel over head halves (2).
Core c handles batch c//2, heads [8*(c%2), 8*(c%2)+8), i.e. output columns
[512*(c%2), 512*(c%2)+512) of out[c//2].

v4 design — all matmul operands bf16; structured so the PE stays saturated
above the ACT exp floor (HAM clock-gates the PE to 1.2 GHz unless its duty
stays high, and re-warming needs a near-fully-busy 3.4us window):

  A) x host-cast to bf16, DMA'd TRANSPOSED via the DMA xbar into
     xT [128, 8*2048]; V projection (v natural, 128-wide per-head slices:
     [v | ones | zero-pad] so LDWEIGHTS gets FWL) overlaps chunk arrivals.
  B) qT/kT [128, 2048] per head-pair g (W stationary, accumulate 8 d_in
     tiles). Only g=0 runs as a phase; g=1..3 are INJECTED one matmul at a
     time into phase C's k-loops as PE filler above the ACT-bound stretches.
  C) per (g, j-block of 512 q, k-tile i), diag offset m = i-4j:
       scores st[:, 512sg+128m:] per head (truncated to causally live cols,
         no mask matmuls)
       pt = exp(st/8) on ACT, one strided ACTIVATE covering both heads
       triangular blocks of both heads *= T (0/1) in one strided GpSimd op
       ctxT[128, 512sg+128m:] += v128.T @ pt (PSUM accumulate, emitted two
       k-tiles late; rows 0-63 ctx, row 64 softmax denominator, rows 65-127
       zero pad)
     tails (cts cast -> PE transposes -> reciprocal -> normalize -> DMA) are
     chunked and paced into the following k-loop together with the injected
     B2 matmuls. Single pool scope end to end: no phase barriers, no PE
     drain, HAM stays warm.

PSUM: st ring 2x2 banks + ctxT 2 + nat 1 + proj ring 1 = 8 banks.
"""

import os
import sys

for _p in ("/opt/trn_rl_repo", "/root/.axon_site/_ro/trn_rl_repo"):
    if os.path.isdir(_p) and _p not in sys.path:
        sys.path.insert(0, _p)

import ml_dtypes
import numpy as np

import concourse.mybir as mybir  # noqa: E402
import concourse.tile as tile  # noqa: E402
from concourse import bacc  # noqa: E402
from concourse.bass_utils import run_bass_kernel_spmd  # noqa: E402

F32 = mybir.dt.float32
BF16 = mybir.dt.bfloat16
BF = ml_dtypes.bfloat16

P = 128
T = 2048
DIN = 1024
DL = 512          # local d_out per core
HL = 8            # local heads
DH = 64
NT = T // P       # 16 t-tiles
NDI = DIN // P    # 8 d_in tiles
SCALE = 1.0 / np.sqrt(DH)

Exp = mybir.ActivationFunctionType.Exp


def _build():
    nc = bacc.Bacc(None, target_bir_lowering=False)
    x = nc.dram_tensor("x", [T, DIN], BF16, kind="ExternalInput")
    wq = nc.dram_tensor("wq", [DIN, DL], BF16, kind="ExternalInput")
    wk = nc.dram_tensor("wk", [DIN, DL], BF16, kind="ExternalInput")
    wv = nc.dram_tensor("wv", [DIN, DL], BF16, kind="ExternalInput")
    ident_d = nc.dram_tensor("ident", [P, P], BF16, kind="ExternalInput")
    tmask_d = nc.dram_tensor("tmask", [P, 2 * P], BF16, kind="ExternalInput")
    out = nc.dram_tensor("out", [T, DL], F32, kind="ExternalOutput")

    w_r = {n: w[:].rearrange("(k p) n -> k p n", p=P) for n, w in
           (("q", wq), ("k", wk), ("v", wv))}
    # out rows 512*j + 128*c + p
    out_r = out[:].rearrange("(j c p) n -> j p c n", j=4, c=4)

    with tile.TileContext(nc) as tc:
        with (
            tc.tile_pool(name="const", bufs=1) as const,
            tc.tile_pool(name="qk", bufs=1) as qk_pool,
            tc.tile_pool(name="v", bufs=1) as v_pool,
            tc.tile_pool(name="xt", bufs=1) as xt_pool,
            tc.tile_pool(name="wvp", bufs=1) as wv_pool,
            tc.tile_pool(name="wqp", bufs=1) as wq_pool,
            tc.tile_pool(name="wkp", bufs=1) as wk_pool,
            tc.tile_pool(name="pt", bufs=6) as pt_pool,
            tc.tile_pool(name="cs", bufs=3) as cs_pool,
            tc.tile_pool(name="o", bufs=3) as o_pool,
            tc.tile_pool(name="ps_pj", bufs=1, space="PSUM") as ps_pj,
            tc.tile_pool(name="ps_s", bufs=2, space="PSUM") as ps_s,
            tc.tile_pool(name="ps_ctx", bufs=1, space="PSUM") as ps_ctx,
            tc.tile_pool(name="ps_nat", bufs=1, space="PSUM") as ps_nat,
        ):
            ident = const.tile([P, P], BF16)
            nc.sync.dma_start(out=ident, in_=ident_d[:])
            tmask2 = const.tile([P, 2 * P], BF16)
            nc.sync.dma_start(out=tmask2, in_=tmask_d[:])
            ones_bf = const.tile([P, HL], BF16)
            nc.vector.memset(ones_bf, 1.0)
            # v tiles: per head a 128-wide slice [v(64) | ones(1) | pad0(63)]
            # (128-wide stationary => FWL fast weight load)
            v_sb = [v_pool.tile([P, HL * P], BF16, tag=f"v{t_}",
                                name=f"v{t_}") for t_ in range(NT)]
            qTs = [qk_pool.tile([P, T], BF16, tag=f"qT{g}", name=f"qT{g}")
                   for g in range(4)]
            kTs = [qk_pool.tile([P, T], BF16, tag=f"kT{g}", name=f"kT{g}")
                   for g in range(4)]
            xT = xt_pool.tile([P, NDI * T], BF16, name="xT")
            wv_t = [wv_pool.tile([P, DL], BF16, tag=f"wv{di}",
                                 name=f"wv{di}") for di in range(NDI)]
            wq_t = [wq_pool.tile([P, DL], BF16, tag=f"wq{di}",
                                 name=f"wq{di}") for di in range(NDI)]
            wk_t = [wk_pool.tile([P, DL], BF16, tag=f"wk{di}",
                                 name=f"wk{di}") for di in range(NDI)]

            # ---- DMAs ----
            # all of x through the xbar transpose unit on the sync queue
            # (single-queue: the xbar is not safe from two queues);
            # weights go on the scalar queue so they land within ~10us.
            for di in range(NDI):
                nc.sync.dma_start(out=wv_t[di], in_=w_r["v"][di])
            for tq in range(4):
                for di in range(NDI):
                    nc.sync.dma_start(
                        out=xT[:, T * di + 512 * tq:T * di + 512 * (tq + 1)],
                        in_=x[:][512 * tq:512 * (tq + 1),
                                 P * di:P * (di + 1)],
                        transpose=True)
            for di in range(NDI):
                nc.sync.dma_start(out=wq_t[di], in_=w_r["q"][di])
            for di in range(NDI):
                nc.sync.dma_start(out=wk_t[di], in_=w_r["k"][di])

            # ---- Phase A: V projection (DMA-paced) ----
            for t_ in range(NT):
                psv = ps_pj.tile([P, DL], F32, tag="pj", name="psv")
                for di in range(NDI):
                    nc.tensor.matmul(
                        psv, xT[:, T * di + P * t_:T * di + P * (t_ + 1)],
                        wv_t[di], start=(di == 0), stop=(di == NDI - 1))
                vt = v_sb[t_][:].rearrange("p (h e) -> p h e", e=P)
                nc.vector.tensor_copy(
                    vt[:, :, 0:DH], psv[:].rearrange("p (h d) -> p h d", d=DH))
                nc.vector.tensor_copy(vt[:, :, DH], ones_bf)
                nc.vector.memset(vt[:, :, DH + 1:P], 0.0)

            # ---- B2 projection emitters (g=0 inline; g>=1 injected) ----
            def b2_chunks(g):
                chunks = []
                for which, w_t, dst in (("q", wq_t, qTs[g]),
                                        ("k", wk_t, kTs[g])):
                    for tb in range(4):
                        state = {}

                        def mm(di, state=state, w_t=w_t, g=g, tb=tb):
                            if di == 0:
                                state["ps"] = ps_pj.tile([P, DL], F32,
                                                         tag="pj", name="pj")
                            nc.tensor.matmul(
                                state["ps"], w_t[di][:, P * g:P * (g + 1)],
                                xT[:, T * di + 512 * tb:
                                   T * di + 512 * (tb + 1)],
                                start=(di == 0), stop=(di == NDI - 1))

                        def cp(state=state, dst=dst, tb=tb):
                            nc.vector.tensor_copy(
                                dst[:, 512 * tb:512 * (tb + 1)], state["ps"])

                        for di in range(NDI):
                            chunks.append(lambda di=di, mm=mm: mm(di))
                        chunks.append(cp)
                        # spacer: give the copy a tile of headroom before the
                        # next group's first matmul needs the PSUM slot back
                        chunks.append(lambda: None)
                return chunks

            for c in b2_chunks(0):
                c()

            # ---- Phase C ----
            Copy = mybir.ActivationFunctionType.Copy

            def tail_chunks(g, j, ctxT, use_act=False):
                chunks = []
                half = {}

                def c_cts(sg):
                    cts = cs_pool.tile([DH + 2, 512], BF16, tag="cts",
                                       name="cts")
                    src_ = ctxT[0:DH + 2, 512 * sg:512 * (sg + 1)]
                    if use_act and sg == 1:
                        nc.scalar.copy(cts, src_)
                    else:
                        nc.vector.tensor_copy(cts, src_)
                    half[sg] = {"cts": cts}

                def c_tr(sg, lo):
                    st_ = half[sg]
                    if "natall" not in half:
                        half["natall"] = ps_nat.tile([P, 8 * (DH + 2)], BF16,
                                                     tag="nat", name="nat")
                    if "nat" not in st_:
                        na = half["natall"]
                        st_["nat"] = na[:, 4 * (DH + 2) * sg:
                                        4 * (DH + 2) * (sg + 1)]
                    nat, cts = st_["nat"], st_["cts"]
                    for c in (lo, lo + 1):
                        nc.tensor.transpose(
                            nat[:, (DH + 2) * c:(DH + 2) * (c + 1)],
                            cts[0:DH + 2, P * c:P * (c + 1)],
                            ident[0:DH + 2, 0:DH + 2])

                def c_rec(sg):
                    st_ = half[sg]
                    rec = o_pool.tile([P, 4], F32, tag="rec", name="rec")
                    nc.vector.reciprocal(
                        rec, st_["nat"][:].rearrange(
                            "p (c e) -> p c e", e=DH + 2)[:, :, DH])
                    st_["rec"] = rec
                    if "ob" not in half:
                        half["ob"] = o_pool.tile([P, 8 * DH], F32, tag="ob",
                                                 name="ob")

                def c_norm(sg, lo):
                    st_ = half[sg]
                    for c in (lo, lo + 1):
                        dst = half["ob"][:, 2 * DH * c + DH * sg:
                                         2 * DH * c + DH * (sg + 1)]
                        src_ = st_["nat"][:, (DH + 2) * c:(DH + 2) * c + DH]
                        if use_act and sg == 1:
                            nc.scalar.activation(dst, src_, Copy,
                                                 scale=st_["rec"][:, c:c + 1])
                        else:
                            nc.vector.tensor_scalar_mul(
                                dst, src_, st_["rec"][:, c:c + 1])

                def c_out():
                    nc.sync.dma_start(
                        out=out_r[j][:, :, 2 * DH * g:2 * DH * (g + 1)],
                        in_=half["ob"][:].rearrange("p (c d) -> p c d",
                                                    d=2 * DH))

                chunks += [lambda: c_cts(0), lambda: c_cts(1)]
                for sg in range(2):
                    chunks += [
                        lambda sg=sg: c_tr(sg, 0),
                        lambda sg=sg: c_tr(sg, 2),
                        lambda sg=sg: c_rec(sg),
                        lambda sg=sg: c_norm(sg, 0),
                        lambda sg=sg: c_norm(sg, 2),
                    ]
                chunks.append(c_out)
                return chunks

            pending = []     # tail chunks of the previous (g, j)
            av_q = []
            carry = None
            for g in range(4):
                b2_pend = b2_chunks(g + 1) if g < 3 else []
                n_tiles_left = sum(4 * j + 4 for j in (3, 0, 2, 1))
                for j in (3, 0, 2, 1):
                    nk = 4 * j + 4
                    ctxT = ps_ctx.tile([P, 1024], F32, tag="cT", name="ctxT")
                    for i in range(nk):
                        m = i - 4 * j
                        c0 = P * m if m > 0 else 0
                        st = ps_s.tile([P, 1024], F32, tag="s", name="st")
                        for sg in range(2):
                            nc.tensor.matmul(
                                st[:, 512 * sg + c0:512 * (sg + 1)],
                                kTs[g][DH * sg:DH * (sg + 1),
                                       P * i:P * (i + 1)],
                                qTs[g][DH * sg:DH * (sg + 1),
                                       512 * j + c0:512 * (j + 1)],
                                start=True, stop=True)
                        pt = pt_pool.tile([P, 1024], BF16, tag="pt",
                                          name="pt")
                        if m < 0:
                            nc.scalar.activation(pt[:, 0:1024], st[:, 0:1024],
                                                 Exp, scale=float(SCALE))
                        else:
                            nc.scalar.activation(
                                pt[:].rearrange("p (s q) -> p s q",
                                                s=2)[:, :, c0:512],
                                st[:].rearrange("p (s q) -> p s q",
                                                s=2)[:, :, c0:512],
                                Exp, scale=float(SCALE))
                        if m >= 0:
                            pm = pt[:].rearrange("p (s q) -> p s q",
                                                 s=2)[:, :, c0:c0 + P]
                            nc.vector.tensor_mul(
                                pm, pm,
                                tmask2[:].rearrange("p (s q) -> p s q", s=2))
                        if carry is not None:
                            carry()
                            carry = None
                        if pending:
                            nflush = -(-len(pending) // (nk - i))
                            for _ in range(nflush):
                                pending.pop(0)()
                        if b2_pend:
                            nb = -(-len(b2_pend) // max(n_tiles_left, 1))
                            for _ in range(min(nb, len(b2_pend))):
                                b2_pend.pop(0)()

                        def av(i=i, pt=pt, ctxT=ctxT, nk=nk, g=g, c0=c0):
                            for sg in range(2):
                                h = 2 * g + sg
                                nc.tensor.matmul(
                                    ctxT[:, 512 * sg + c0:512 * (sg + 1)],
                                    v_sb[i][:, P * h:P * (h + 1)],
                                    pt[:, 512 * sg + c0:512 * (sg + 1)],
                                    start=(i == 0), stop=(i == nk - 1),
                                    skip_group_check=True)
                        av_q.append(av)
                        if len(av_q) > 3:
                            av_q.pop(0)()
                        n_tiles_left -= 1
                    while len(av_q) > 1:
                        av_q.pop(0)()
                    carry = av_q.pop(0)
                    for c in pending:
                        c()
                    pending = tail_chunks(g, j, ctxT,
                                          use_act=(g == 3 and j == 1))
                for c in b2_pend:
                    c()
            if carry is not None:
                carry()
            for c in pending:
                c()
    nc.compile()
    return nc


_NC = None


def _get_nc():
    global _NC
    if _NC is None:
        _NC = _build()
    return _NC


_IDENT = np.eye(P, dtype=np.float32).astype(BF)
# T[p, u] = 1 where in-block column u (query) >= partition p (key); two
# copies side by side so one strided gpsimd op masks both heads
_T1 = (np.arange(P)[None, :] >= np.arange(P)[:, None]).astype(np.float32)
_TMASK = np.concatenate([_T1, _T1], axis=1).astype(BF)


def run(inputs, **spmd_kwargs):
    x, W_q, W_k, W_v = (inputs["x"], inputs["W_q"], inputs["W_k"], inputs["W_v"])
    nc = _get_nc()
    in_maps = []
    for c in range(8):
        b, half = divmod(c, 2)
        sl = slice(DL * half, DL * half + DL)
        in_maps.append({
            "x": np.ascontiguousarray(np.asarray(x[b]).astype(BF)),
            "wq": np.ascontiguousarray(np.asarray(W_q[:, sl]).astype(BF)),
            "wk": np.ascontiguousarray(np.asarray(W_k[:, sl]).astype(BF)),
            "wv": np.ascontiguousarray(np.asarray(W_v[:, sl]).astype(BF)),
            "ident": _IDENT,
            "tmask": _TMASK,
        })
    res = run_bass_kernel_spmd(nc, in_maps, core_ids=list(range(8)), **spmd_kwargs)
    B = x.shape[0]
    full = np.empty((B, T, 2 * DL), dtype=np.float32)
    for c in range(8):
        b, half = divmod(c, 2)
        full[b][:, DL * half:DL * half + DL] = res.results[c]["out"]
    return full, res


def kernel(**inputs):
    return run(inputs)[0]


if __name__ == "__main__":
    rng = np.random.default_rng(0)
    ins = {
        "x": rng.standard_normal((4, T, DIN), dtype=np.float32),
        "W_q": (rng.random((DIN, 2 * DL), dtype=np.float32) - 0.5) / 16,
        "W_k": (rng.random((DIN, 2 * DL), dtype=np.float32) - 0.5) / 16,
        "W_v": (rng.random((DIN, 2 * DL), dtype=np.float32) - 0.5) / 16,
    }
    o = kernel(**ins)
    print("ran ok", o.shape, o.dtype)
